# revision 11
# baseline (speedup 1.0000x reference)
"""Trainium2 Bass kernel for nn_CALayer (FFT-magnitude channel attention).

Math per (b, c) image X [256, 256] (real):
  F(p, q) = 2D DFT;  y[b,c] = mean over the centered (fftshifted) 100x100
  low-frequency crop of |F|;  s = sigmoid(w2 @ relu(w1 @ y + b1) + b2);
  out = x * s[:, :, None, None].

Implementation: DFT-as-matmul with Hermitian reduction. Since X is real,
|F(-p,-q)| = |F(p,q)|, so only p in 0..50 (51 rows) and q in -50..50
(101 cols) of the spectrum are computed, and the crop sum over
p,q in [-50, 49]^2 is recovered as two q-window sums:
  S = sum_{q in -50..49} sum_{p in 0..49} |F| + sum_{q in -49..50} sum_{p in 1..50} |F|.

Memory-regime design (v2):
  - x is host-pre-transposed to [b, p, c, k, w] (h = 2p + k) so every
    input DMA descriptor moves 16 KiB contiguous per partition.
  - x lands in SBUF as f32, is converted once to an fp16 resident copy
    (both batches fully resident: 16 group tiles x 8 KiB/partition).
    The fp16 copy feeds the FFT matmuls AND is scaled in place and
    DMA'd out as fp16 (output HBM traffic halved); the host converts
    back to f32. Total per-core HBM traffic 50.3 MB ~= 141 us roofline.
  - input DMAs ride the sync-engine HW queue, output DMAs the
    activation-engine HW queue, so neither blocks the other, and the
    input stream never waits on batch-0 writeback (full residency).

Dataflow per unit (batch b, channel-group g of 8):
  - step A (PE): U^T[w, n] = X^T @ Wu per channel (h = 2p+k indexing)
  - step B (PE): F^T[q, (ch, p)] = Wv^T @ U^T  (4 real matmuls per part)
  - mag (ACT/DVE): sqrt(Fr^2 + Fi^2) -> fp16
  - crop sum: both q-window indicator matmuls accumulate into one PSUM
    row (PE), one windowed free-dim reduce (DVE) -> y[1, 64] slice
  - after 8 units: SE block on-device; s broadcast via ones-matmul
  - in-place fp16 per-channel scale of the resident tiles, DMA out

Sharding: pure data parallel over batch: core i handles batches 2i, 2i+1.
"""

import os
import sys

for _p in (
    "/root/.axon_site",
    "/root/.axon_site/_ro/trn_rl_repo",
    "/root/.axon_site/_ro/pypackages",
    "/opt/trn_rl_repo",
):
    if os.path.isdir(_p) and _p not in sys.path:
        sys.path.append(_p)

import numpy as np

import concourse.bacc as bacc
import concourse.mybir as mybir
import concourse.tile as tile
from concourse.bass_utils import run_bass_kernel_spmd

N_CORES = 8
B, C, H, W = 16, 64, 256, 256
BPC = B // N_CORES  # batches per core
CROP = 50
NP_ = 51   # p = 0..50
NQ = 101   # q = -50..50
GS = 8     # channels per group
NG = C // GS
NU = 104   # u cols per w-half: 51 cos + pad + 51 (-sin) + pad
F32 = mybir.dt.float32
F16 = mybir.dt.float16
AF = mybir.ActivationFunctionType


def _build_consts(w1, b1, w2, b2):
    h_idx = np.arange(H)
    ang_p = 2 * np.pi * np.outer(h_idx, np.arange(NP_)) / H
    wu = np.zeros((H, NU), np.float32)
    wu[:, 0:NP_] = np.cos(ang_p)             # cols 0..50, col 51 zero pad
    wu[:, 52:52 + NP_] = -np.sin(ang_p)      # cols 52..102, col 103 zero pad
    wu2 = wu.reshape(128, 2, NU)             # [p, k, n] with h = 2p + k
    ang_q = 2 * np.pi * np.outer(h_idx, np.arange(-CROP, CROP + 1)) / W
    cq = np.cos(ang_q).astype(np.float32)
    sq = np.sin(ang_q).astype(np.float32)
    wv = np.concatenate([cq, sq, -sq], axis=1)      # [256, 303]
    wv2 = wv.reshape(2, 128, 303).transpose(1, 0, 2)  # [p, k, :] with w = 128k + p
    r1 = np.zeros((NQ, 2), np.float32)
    r1[0:100, 0] = 1.0  # q in -50..49
    r1[1:101, 1] = 1.0  # q in -49..50
    return {
        "wu": np.ascontiguousarray(wu2).astype(np.float16),
        "wv": np.ascontiguousarray(wv2).astype(np.float16),
        "r1ind": r1.astype(np.float16),
        "id1": np.ones((1, 1), np.float32),
        "ones128": np.ones((1, 128), np.float32),
        "w1t": np.ascontiguousarray(w1.T.astype(np.float32) / 1e4),  # fold /10000
        "b1c": np.ascontiguousarray(b1.astype(np.float32).reshape(-1, 1)),
        "w2t": np.ascontiguousarray(w2.T.astype(np.float32)),
        "b2r": np.ascontiguousarray(b2.astype(np.float32).reshape(1, -1)),
    }


def _build_nc():
    nc = bacc.Bacc("TRN2", target_bir_lowering=False, debug=False)
    # x pre-transposed AND pre-converted to fp16 on host:
    # [b, p, c, k, w] with h = 2p + k
    x_d = nc.dram_tensor("x", [BPC, 128, C, 2, W], F16, kind="ExternalInput").ap()
    out_d = nc.dram_tensor("out", [BPC, 128, C, 2, W], F16, kind="ExternalOutput").ap()
    wu_d = nc.dram_tensor("wu", [128, 2, NU], F16, kind="ExternalInput").ap()
    wv_d = nc.dram_tensor("wv", [128, 2, 303], F16, kind="ExternalInput").ap()
    r1_d = nc.dram_tensor("r1ind", [NQ, 2], F16, kind="ExternalInput").ap()
    id1_d = nc.dram_tensor("id1", [1, 1], F32, kind="ExternalInput").ap()
    ones128_d = nc.dram_tensor("ones128", [1, 128], F32, kind="ExternalInput").ap()
    w1t_d = nc.dram_tensor("w1t", [C, 4], F32, kind="ExternalInput").ap()
    b1c_d = nc.dram_tensor("b1c", [4, 1], F32, kind="ExternalInput").ap()
    w2t_d = nc.dram_tensor("w2t", [4, C], F32, kind="ExternalInput").ap()
    b2r_d = nc.dram_tensor("b2r", [1, C], F32, kind="ExternalInput").ap()

    with tile.TileContext(nc) as tc:
        with (
            tc.tile_pool(name="consts", bufs=1) as cpool,
            tc.tile_pool(name="xr", bufs=2 * NG) as xpool,
            tc.tile_pool(name="work", bufs=2) as wpool,
            tc.tile_pool(name="psA", bufs=2, space="PSUM") as pA,
            tc.tile_pool(name="psB", bufs=2, space="PSUM") as pB,
            tc.tile_pool(name="psS", bufs=1, space="PSUM") as pS,
        ):
            wu_sb = cpool.tile([128, 2, NU], F16, name="wu_sb")
            nc.sync.dma_start(wu_sb[:], wu_d[:])
            wv_sb = cpool.tile([128, 2, 303], F16, name="wv_sb")
            nc.sync.dma_start(wv_sb[:], wv_d[:])
            r1_sb = cpool.tile([NQ, 2], F16, name="r1_sb")
            nc.sync.dma_start(r1_sb[:], r1_d[:])
            id1_sb = cpool.tile([1, 1], F32, name="id1_sb")
            nc.sync.dma_start(id1_sb[:], id1_d[:])
            ones128_sb = cpool.tile([1, 128], F32, name="ones128_sb")
            nc.sync.dma_start(ones128_sb[:], ones128_d[:])
            w1t_sb = cpool.tile([C, 4], F32, name="w1t_sb")
            nc.sync.dma_start(w1t_sb[:], w1t_d[:])
            b1c_sb = cpool.tile([4, 1], F32, name="b1c_sb")
            nc.sync.dma_start(b1c_sb[:], b1c_d[:])
            w2t_sb = cpool.tile([4, C], F32, name="w2t_sb")
            nc.sync.dma_start(w2t_sb[:], w2t_d[:])
            b2r_sb = cpool.tile([1, C], F32, name="b2r_sb")
            nc.sync.dma_start(b2r_sb[:], b2r_d[:])

            xrs = {}
            ys = {}
            sbs = {}

            def emit_unit(b, g):
                xr = xpool.tile([128, GS, 2, W], F16, name="xr", tag="xr")
                nc.sync.dma_start(xr[:], x_d[b, :, GS * g:GS * (g + 1)])
                xrs[(b, g)] = xr

                if g == 0:
                    ys[b] = wpool.tile([1, C], F32, name="y_sb", tag="y")
                y_sb = ys[b]

                # ---- step A: U^T = X^T @ Wu per channel (both w-chunks),
                # PSUM batched over pairs of channels to halve copy count
                u_sb = wpool.tile([128, GS, 2, NU], F16, name="u_sb", tag="u")
                for jb in range(GS // 2):
                    psA = pA.tile([128, 2, 2, NU], F32, name="psA", tag="uA")
                    for jj in range(2):
                        j = 2 * jb + jj
                        for wk in range(2):
                            for kk in range(2):
                                nc.tensor.matmul(
                                    psA[:, jj, wk, :],
                                    xr[:, j, kk, 128 * wk:128 * (wk + 1)],
                                    wu_sb[:, kk, :],
                                    start=(kk == 0),
                                    stop=(kk == 1),
                                )
                    dst = u_sb[:, 2 * jb:2 * jb + 2]
                    if jb == 1:
                        nc.scalar.copy(dst, psA[:])
                    else:
                        nc.vector.tensor_copy(dst, psA[:])

                # ---- step B: F^T[q, (ch, p)] with complex arithmetic
                psB = pB.tile([NQ, 1024], F32, name="psB", tag="fB")
                fr = psB[:, 0:416]
                fi = psB[:, 512:928]
                fr_terms, fi_terms = [], []
                for k in range(2):
                    ur = u_sb[:, :, k, 0:52]
                    ui = u_sb[:, :, k, 52:104]
                    ck = wv_sb[:, k, 0:101]
                    sk = wv_sb[:, k, 101:202]
                    snk = wv_sb[:, k, 202:303]
                    fr_terms += [(ck, ur), (sk, ui)]
                    fi_terms += [(ck, ui), (snk, ur)]
                for i, (lhsT, rhs) in enumerate(fr_terms):
                    nc.tensor.matmul(fr, lhsT, rhs, start=(i == 0), stop=(i == 3))
                for i, (lhsT, rhs) in enumerate(fi_terms):
                    nc.tensor.matmul(fi, lhsT, rhs, start=(i == 0), stop=(i == 3))

                # ---- |F| = sqrt(Fr^2 + Fi^2), fp16: one ACT square over the
                # (fr, fi) pair, add on gpsimd (SBUF only), sqrt on ACT
                m2p = wpool.tile([NQ, 2, 416], F32, name="m2p", tag="m2p")
                pair = psB.rearrange("p (a x) -> p a x", a=2)[:, :, 0:416]
                nc.scalar.square(m2p[:], pair)
                nc.gpsimd.tensor_add(m2p[:, 0], m2p[:, 0], m2p[:, 1])
                mag = wpool.tile([NQ, 416], F16, name="mag", tag="mag")
                nc.scalar.sqrt(mag[:], m2p[:, 0])

                # ---- crop sum: both q-window matmuls accumulate in PSUM,
                # then one windowed free-dim reduce -> y row slice
                mag3 = mag.rearrange("p (c x) -> p c x", c=GS)
                g2 = pS.tile([1, 400], F32, name="g2", tag="G")
                nc.tensor.matmul(
                    g2[:], r1_sb[:, 0:1], mag3[:, :, 0:50], start=True, stop=False
                )
                nc.tensor.matmul(
                    g2[:], r1_sb[:, 1:2], mag3[:, :, 1:51], start=False, stop=True
                )
                ga = g2.rearrange("p (c x) -> p c x", c=GS)
                nc.vector.reduce_sum(
                    y_sb[0:1, GS * g:GS * (g + 1)], ga, axis=mybir.AxisListType.X
                )

            def emit_se(b):
                # ---- SE block (y is pre-divided by 1e4 via w1t folding)
                y_sb = ys[b]
                yT_ps = pS.tile([C, 1], F32, name="yT_ps", tag="se")
                nc.tensor.transpose(yT_ps[:], y_sb[:], id1_sb[:])
                y_col = wpool.tile([C, 1], F32, name="y_col", tag="se2")
                nc.scalar.copy(y_col[:], yT_ps[:])
                h_ps = pS.tile([4, 1], F32, name="h_ps", tag="se")
                nc.tensor.matmul(h_ps[:], w1t_sb[:], y_col[:], start=True, stop=True)
                h_sb = wpool.tile([4, 1], F32, name="h_sb", tag="se3")
                nc.scalar.activation(h_sb[:], h_ps[:], AF.Relu, bias=b1c_sb[:])
                sarg_ps = pS.tile([1, C], F32, name="sarg_ps", tag="se")
                nc.tensor.matmul(sarg_ps[:], h_sb[:], w2t_sb[:], start=True, stop=True)
                sarg_sb = wpool.tile([1, C], F32, name="sarg_sb", tag="se4")
                nc.vector.tensor_add(sarg_sb[:], sarg_ps[:], b2r_sb[:])
                s_row = wpool.tile([1, C], F32, name="s_row", tag="se5")
                nc.scalar.activation(s_row[:], sarg_sb[:], AF.Sigmoid)
                sb_ps = pS.tile([128, C], F32, name="sb_ps", tag="se")
                nc.tensor.matmul(
                    sb_ps[:], ones128_sb[:], s_row[:], start=True, stop=True
                )
                s_b = wpool.tile([128, C], F32, name="s_b", tag="se6")
                nc.vector.tensor_copy(s_b[:], sb_ps[:])
                s_h = wpool.tile([128, C], F16, name="s_h", tag="se7")
                nc.scalar.copy(s_h[:], sb_ps[:])
                sbs[b] = (s_b, s_h)

            def emit_scale(b, g):
                # in-place fp16 scale split 3/3/2 over DVE / gpsimd / ACT
                xr = xrs[(b, g)]
                s_b, s_h = sbs[b]
                sg1 = (
                    s_h[:, GS * g:GS * g + 3]
                    .unsqueeze(-1).unsqueeze(-1).broadcast_to([128, 3, 2, W])
                )
                nc.vector.tensor_mul(xr[:, 0:3], xr[:, 0:3], sg1)
                sg2 = (
                    s_h[:, GS * g + 3:GS * g + 6]
                    .unsqueeze(-1).unsqueeze(-1).broadcast_to([128, 3, 2, W])
                )
                nc.gpsimd.tensor_mul(xr[:, 3:6], xr[:, 3:6], sg2)
                for j in range(6, GS):
                    sc = s_b[:, GS * g + j:GS * g + j + 1]
                    nc.scalar.mul(xr[:, j], xr[:, j], sc)

            def emit_out(b, g, eng):
                eng.dma_start(out_d[b, :, GS * g:GS * (g + 1)], xrs[(b, g)][:])

            # Schedule: batch-0 SE right after its last unit; batch-0 scales
            # and writebacks interleaved with batch-1 units (lagged so the
            # in-order engine streams never stall on not-yet-ready sems);
            # batch-0 outs ride the ACT HW queue (overlap b1 input), batch-1
            # outs ride the sync queue (input long done by then).
            units = [(b, g) for b in range(BPC) for g in range(NG)]
            for u, (b, g) in enumerate(units):
                emit_unit(b, g)
                if u == NG - 1:
                    emit_se(0)
                if NG + 1 <= u <= NG + 4:
                    gg = 2 * (u - NG - 1)
                    emit_scale(0, gg)
                    emit_scale(0, gg + 1)
                if NG + 3 <= u <= NG + 6:
                    gg = 2 * (u - NG - 3)
                    emit_out(0, gg, nc.scalar)
                    emit_out(0, gg + 1, nc.scalar)
            emit_se(1)
            for g in range(NG):
                emit_scale(1, g)
                emit_out(1, g, nc.sync)

    nc.compile()
    return nc


_NC = None


def _get_nc():
    global _NC
    if _NC is None:
        _NC = _build_nc()
    return _NC


def _execute(inputs, trace=False):
    x = np.asarray(inputs["x"], dtype=np.float32)
    consts = _build_consts(
        np.asarray(inputs["w1"]), np.asarray(inputs["b1"]),
        np.asarray(inputs["w2"]), np.asarray(inputs["b2"]),
    )
    in_maps = []
    for i in range(N_CORES):
        xs = x[BPC * i:BPC * (i + 1)]
        # [b, c, (p k), w] -> [b, p, c, k, w]  (h = 2p + k), fp16
        xs = np.ascontiguousarray(
            xs.reshape(BPC, C, 128, 2, W).transpose(0, 2, 1, 3, 4),
            dtype=np.float16,
        )
        m = {"x": xs}
        m.update(consts)
        in_maps.append(m)
    nc = _get_nc()
    res = run_bass_kernel_spmd(nc, in_maps, core_ids=list(range(N_CORES)), trace=trace)
    outs = []
    for i in range(N_CORES):
        o = res.results[i]["out"]  # [b, p, c, k, w] fp16
        o = o.transpose(0, 2, 1, 3, 4).reshape(BPC, C, H, W).astype(np.float32)
        outs.append(o)
    out = np.concatenate(outs, axis=0)
    return out, res


def kernel(x, w1, b1, w2, b2):
    out, _ = _execute({"x": x, "w1": w1, "b1": b1, "w2": w2, "b2": b2}, trace=False)
    return out


# revision 16
# speedup vs baseline: 1.0150x; 1.0150x over previous
"""Trainium2 Bass kernel for nn_CALayer (FFT-magnitude channel attention).

Math per (b, c) image X [256, 256] (real):
  F(p, q) = 2D DFT;  y[b,c] = mean over the centered (fftshifted) 100x100
  low-frequency crop of |F|;  s = sigmoid(w2 @ relu(w1 @ y + b1) + b2);
  out = x * s[:, :, None, None].

Implementation: DFT-as-matmul with Hermitian reduction. Since X is real,
|F(-p,-q)| = |F(p,q)|, so only p in 0..50 (51 rows) and q in -50..50
(101 cols) of the spectrum are computed, and the crop sum over
p,q in [-50, 49]^2 is recovered as two q-window sums:
  S = sum_{q in -50..49} sum_{p in 0..49} |F| + sum_{q in -49..50} sum_{p in 1..50} |F|.

Memory-regime design (v2):
  - x is host-pre-transposed to [b, p, c, k, w] (h = 2p + k) so every
    input DMA descriptor moves 16 KiB contiguous per partition.
  - x lands in SBUF as f32, is converted once to an fp16 resident copy
    (both batches fully resident: 16 group tiles x 8 KiB/partition).
    The fp16 copy feeds the FFT matmuls AND is scaled in place and
    DMA'd out as fp16 (output HBM traffic halved); the host converts
    back to f32. Total per-core HBM traffic 50.3 MB ~= 141 us roofline.
  - input DMAs ride the sync-engine HW queue, output DMAs the
    activation-engine HW queue, so neither blocks the other, and the
    input stream never waits on batch-0 writeback (full residency).

Dataflow per unit (batch b, channel-group g of 8):
  - step A (PE): U^T[w, n] = X^T @ Wu per channel (h = 2p+k indexing)
  - step B (PE): F^T[q, (ch, p)] = Wv^T @ U^T  (4 real matmuls per part)
  - mag (ACT/DVE): sqrt(Fr^2 + Fi^2) -> fp16
  - crop sum: both q-window indicator matmuls accumulate into one PSUM
    row (PE), one windowed free-dim reduce (DVE) -> y[1, 64] slice
  - after 8 units: SE block on-device; s broadcast via ones-matmul
  - in-place fp16 per-channel scale of the resident tiles, DMA out

Sharding: pure data parallel over batch: core i handles batches 2i, 2i+1.
"""

import os
import sys

for _p in (
    "/root/.axon_site",
    "/root/.axon_site/_ro/trn_rl_repo",
    "/root/.axon_site/_ro/pypackages",
    "/opt/trn_rl_repo",
):
    if os.path.isdir(_p) and _p not in sys.path:
        sys.path.append(_p)

import numpy as np

import concourse.bacc as bacc
import concourse.mybir as mybir
import concourse.tile as tile
from concourse.bass_utils import run_bass_kernel_spmd

N_CORES = 8
B, C, H, W = 16, 64, 256, 256
BPC = B // N_CORES  # batches per core
CROP = 50
NP_ = 51   # p = 0..50
NQ = 101   # q = -50..50
GS = 8     # channels per group
NG = C // GS
NU = 104   # u cols per w-half: 51 cos + pad + 51 (-sin) + pad
F32 = mybir.dt.float32
F16 = mybir.dt.float16
AF = mybir.ActivationFunctionType


def _build_consts(w1, b1, w2, b2):
    h_idx = np.arange(H)
    ang_p = 2 * np.pi * np.outer(h_idx, np.arange(NP_)) / H
    wu = np.zeros((H, NU), np.float32)
    wu[:, 0:NP_] = np.cos(ang_p)             # cols 0..50, col 51 zero pad
    wu[:, 52:52 + NP_] = -np.sin(ang_p)      # cols 52..102, col 103 zero pad
    wu2 = wu.reshape(128, 2, NU)             # [p, k, n] with h = 2p + k
    ang_q = 2 * np.pi * np.outer(h_idx, np.arange(-CROP, CROP + 1)) / W
    cq = np.cos(ang_q).astype(np.float32)
    sq = np.sin(ang_q).astype(np.float32)
    wv = np.concatenate([cq, sq, -sq], axis=1)      # [256, 303]
    wv2 = wv.reshape(2, 128, 303).transpose(1, 0, 2)  # [p, k, :] with w = 128k + p
    r1 = np.zeros((NQ, 2), np.float32)
    r1[0:100, 0] = 1.0  # q in -50..49
    r1[1:101, 1] = 1.0  # q in -49..50
    return {
        "wu": np.ascontiguousarray(wu2).astype(np.float16),
        "wv": np.ascontiguousarray(wv2).astype(np.float16),
        "r1ind": r1.astype(np.float16),
        "id1": np.ones((1, 1), np.float32),
        "ones128": np.ones((1, 128), np.float32),
        "w1t": np.ascontiguousarray(w1.T.astype(np.float32) / 1e4),  # fold /10000
        "b1c": np.ascontiguousarray(b1.astype(np.float32).reshape(-1, 1)),
        "w2t": np.ascontiguousarray(w2.T.astype(np.float32)),
        "b2r": np.ascontiguousarray(b2.astype(np.float32).reshape(1, -1)),
    }


def _build_nc():
    nc = bacc.Bacc("TRN2", target_bir_lowering=False, debug=False)
    # x pre-transposed AND pre-converted to fp16 on host:
    # [b, p, c, k, w] with h = 2p + k
    x_d = nc.dram_tensor("x", [BPC, 128, C, 2, W], F16, kind="ExternalInput").ap()
    out_d = nc.dram_tensor("out", [BPC, 128, C, 2, W], F16, kind="ExternalOutput").ap()
    wu_d = nc.dram_tensor("wu", [128, 2, NU], F16, kind="ExternalInput").ap()
    wv_d = nc.dram_tensor("wv", [128, 2, 303], F16, kind="ExternalInput").ap()
    r1_d = nc.dram_tensor("r1ind", [NQ, 2], F16, kind="ExternalInput").ap()
    id1_d = nc.dram_tensor("id1", [1, 1], F32, kind="ExternalInput").ap()
    ones128_d = nc.dram_tensor("ones128", [1, 128], F32, kind="ExternalInput").ap()
    w1t_d = nc.dram_tensor("w1t", [C, 4], F32, kind="ExternalInput").ap()
    b1c_d = nc.dram_tensor("b1c", [4, 1], F32, kind="ExternalInput").ap()
    w2t_d = nc.dram_tensor("w2t", [4, C], F32, kind="ExternalInput").ap()
    b2r_d = nc.dram_tensor("b2r", [1, C], F32, kind="ExternalInput").ap()

    with tile.TileContext(nc) as tc:
        with (
            tc.tile_pool(name="consts", bufs=1) as cpool,
            tc.tile_pool(name="xr", bufs=2 * NG) as xpool,
            tc.tile_pool(name="work", bufs=2) as wpool,
            tc.tile_pool(name="psA", bufs=2, space="PSUM") as pA,
            tc.tile_pool(name="psB", bufs=2, space="PSUM") as pB,
            tc.tile_pool(name="psS", bufs=1, space="PSUM") as pS,
        ):
            wu_sb = cpool.tile([128, 2, NU], F16, name="wu_sb")
            nc.sync.dma_start(wu_sb[:], wu_d[:])
            wv_sb = cpool.tile([128, 2, 303], F16, name="wv_sb")
            nc.sync.dma_start(wv_sb[:], wv_d[:])
            r1_sb = cpool.tile([NQ, 2], F16, name="r1_sb")
            nc.sync.dma_start(r1_sb[:], r1_d[:])
            id1_sb = cpool.tile([1, 1], F32, name="id1_sb")
            nc.sync.dma_start(id1_sb[:], id1_d[:])
            ones128_sb = cpool.tile([1, 128], F32, name="ones128_sb")
            nc.sync.dma_start(ones128_sb[:], ones128_d[:])
            w1t_sb = cpool.tile([C, 4], F32, name="w1t_sb")
            nc.sync.dma_start(w1t_sb[:], w1t_d[:])
            b1c_sb = cpool.tile([4, 1], F32, name="b1c_sb")
            nc.sync.dma_start(b1c_sb[:], b1c_d[:])
            w2t_sb = cpool.tile([4, C], F32, name="w2t_sb")
            nc.sync.dma_start(w2t_sb[:], w2t_d[:])
            b2r_sb = cpool.tile([1, C], F32, name="b2r_sb")
            nc.sync.dma_start(b2r_sb[:], b2r_d[:])

            xrs = {}
            ys = {}
            sbs = {}

            def emit_unit(b, g):
                xr = xpool.tile([128, GS, 2, W], F16, name="xr", tag="xr")
                nc.sync.dma_start(xr[:], x_d[b, :, GS * g:GS * (g + 1)])
                xrs[(b, g)] = xr

                if g == 0:
                    ys[b] = wpool.tile([1, C], F32, name="y_sb", tag="y")
                y_sb = ys[b]

                # ---- step A: U^T = X^T @ Wu per channel (both w-chunks),
                # PSUM batched over pairs of channels to halve copy count
                u_sb = wpool.tile([128, GS, 2, NU], F16, name="u_sb", tag="u")
                for jb in range(GS // 2):
                    psA = pA.tile([128, 2, 2, NU], F32, name="psA", tag="uA")
                    for jj in range(2):
                        j = 2 * jb + jj
                        for wk in range(2):
                            for kk in range(2):
                                nc.tensor.matmul(
                                    psA[:, jj, wk, :],
                                    xr[:, j, kk, 128 * wk:128 * (wk + 1)],
                                    wu_sb[:, kk, :],
                                    start=(kk == 0),
                                    stop=(kk == 1),
                                )
                    dst = u_sb[:, 2 * jb:2 * jb + 2]
                    if jb == 3:
                        nc.scalar.copy(dst, psA[:])
                    else:
                        nc.vector.tensor_copy(dst, psA[:])

                # ---- step B: F^T[q, (ch, p)] with complex arithmetic
                psB = pB.tile([NQ, 1024], F32, name="psB", tag="fB")
                fr = psB[:, 0:416]
                fi = psB[:, 512:928]
                fr_terms, fi_terms = [], []
                for k in range(2):
                    ur = u_sb[:, :, k, 0:52]
                    ui = u_sb[:, :, k, 52:104]
                    ck = wv_sb[:, k, 0:101]
                    sk = wv_sb[:, k, 101:202]
                    snk = wv_sb[:, k, 202:303]
                    fr_terms += [(ck, ur), (sk, ui)]
                    fi_terms += [(ck, ui), (snk, ur)]
                for i, (lhsT, rhs) in enumerate(fr_terms):
                    nc.tensor.matmul(fr, lhsT, rhs, start=(i == 0), stop=(i == 3))
                for i, (lhsT, rhs) in enumerate(fi_terms):
                    nc.tensor.matmul(fi, lhsT, rhs, start=(i == 0), stop=(i == 3))

                # ---- |F| = sqrt(Fr^2 + Fi^2), fp16: one ACT square over the
                # (fr, fi) pair, add on gpsimd (SBUF only), sqrt on ACT
                m2p = wpool.tile([NQ, 2, 416], F32, name="m2p", tag="m2p")
                pair = psB.rearrange("p (a x) -> p a x", a=2)[:, :, 0:416]
                nc.scalar.square(m2p[:], pair)
                nc.vector.tensor_add(m2p[:, 0], m2p[:, 0], m2p[:, 1])
                mag = wpool.tile([NQ, 416], F16, name="mag", tag="mag")
                nc.scalar.sqrt(mag[:], m2p[:, 0])

                # ---- crop sum: both q-window matmuls accumulate in PSUM,
                # then one windowed free-dim reduce -> y row slice
                mag3 = mag.rearrange("p (c x) -> p c x", c=GS)
                g2 = pS.tile([1, 400], F32, name="g2", tag="G")
                nc.tensor.matmul(
                    g2[:], r1_sb[:, 0:1], mag3[:, :, 0:50], start=True, stop=False
                )
                nc.tensor.matmul(
                    g2[:], r1_sb[:, 1:2], mag3[:, :, 1:51], start=False, stop=True
                )
                ga = g2.rearrange("p (c x) -> p c x", c=GS)
                nc.vector.reduce_sum(
                    y_sb[0:1, GS * g:GS * (g + 1)], ga, axis=mybir.AxisListType.X
                )

            def emit_se(b):
                # ---- SE block (y is pre-divided by 1e4 via w1t folding)
                y_sb = ys[b]
                yT_ps = pS.tile([C, 1], F32, name="yT_ps", tag="se")
                nc.tensor.transpose(yT_ps[:], y_sb[:], id1_sb[:])
                y_col = wpool.tile([C, 1], F32, name="y_col", tag="se2")
                nc.scalar.copy(y_col[:], yT_ps[:])
                h_ps = pS.tile([4, 1], F32, name="h_ps", tag="se")
                nc.tensor.matmul(h_ps[:], w1t_sb[:], y_col[:], start=True, stop=True)
                h_sb = wpool.tile([4, 1], F32, name="h_sb", tag="se3")
                nc.scalar.activation(h_sb[:], h_ps[:], AF.Relu, bias=b1c_sb[:])
                sarg_ps = pS.tile([1, C], F32, name="sarg_ps", tag="se")
                nc.tensor.matmul(sarg_ps[:], h_sb[:], w2t_sb[:], start=True, stop=True)
                sarg_sb = wpool.tile([1, C], F32, name="sarg_sb", tag="se4")
                nc.vector.tensor_add(sarg_sb[:], sarg_ps[:], b2r_sb[:])
                s_row = wpool.tile([1, C], F32, name="s_row", tag="se5")
                nc.scalar.activation(s_row[:], sarg_sb[:], AF.Sigmoid)
                sb_ps = pS.tile([128, C], F32, name="sb_ps", tag="se")
                nc.tensor.matmul(
                    sb_ps[:], ones128_sb[:], s_row[:], start=True, stop=True
                )
                s_b = wpool.tile([128, C], F32, name="s_b", tag="se6")
                nc.vector.tensor_copy(s_b[:], sb_ps[:])
                s_h = wpool.tile([128, C], F16, name="s_h", tag="se7")
                nc.scalar.copy(s_h[:], sb_ps[:])
                sbs[b] = (s_b, s_h)

            def emit_scale(b, g):
                # in-place fp16 scale: 5 channels as flat per-channel
                # tensor_scalar on DVE (eligible for the packed fast path),
                # 3 channels as one broadcast multiply on gpsimd
                xr = xrs[(b, g)]
                s_b, s_h = sbs[b]
                xf = xr.rearrange("p c k w -> p c (k w)")
                for j in range(5):
                    sc = s_b[:, GS * g + j:GS * g + j + 1]
                    nc.vector.tensor_scalar_mul(xf[:, j], xf[:, j], sc)
                sg2 = (
                    s_h[:, GS * g + 5:GS * g + 8]
                    .unsqueeze(-1).unsqueeze(-1).broadcast_to([128, 3, 2, W])
                )
                nc.gpsimd.tensor_mul(xr[:, 5:8], xr[:, 5:8], sg2)

            def emit_out(b, g, eng):
                eng.dma_start(out_d[b, :, GS * g:GS * (g + 1)], xrs[(b, g)][:])

            # Schedule: batch-0 SE right after its last unit; batch-0 scales
            # and writebacks interleaved with batch-1 units (lagged so the
            # in-order engine streams never stall on not-yet-ready sems);
            # batch-0 outs ride the ACT HW queue (overlap b1 input), batch-1
            # outs ride the sync queue (input long done by then).
            units = [(b, g) for b in range(BPC) for g in range(NG)]
            for u, (b, g) in enumerate(units):
                emit_unit(b, g)
                if u == NG - 1:
                    emit_se(0)
                if NG + 1 <= u <= NG + 4:
                    gg = 2 * (u - NG - 1)
                    emit_scale(0, gg)
                    emit_scale(0, gg + 1)
                if NG + 3 <= u <= NG + 6:
                    gg = 2 * (u - NG - 3)
                    emit_out(0, gg, nc.scalar)
                    emit_out(0, gg + 1, nc.scalar)
            emit_se(1)
            for g in range(NG):
                emit_scale(1, g)
                emit_out(1, g, nc.scalar)

    nc.compile()
    return nc


_NC = None


def _get_nc():
    global _NC
    if _NC is None:
        _NC = _build_nc()
    return _NC


def _execute(inputs, trace=False):
    x = np.asarray(inputs["x"], dtype=np.float32)
    consts = _build_consts(
        np.asarray(inputs["w1"]), np.asarray(inputs["b1"]),
        np.asarray(inputs["w2"]), np.asarray(inputs["b2"]),
    )
    in_maps = []
    for i in range(N_CORES):
        xs = x[BPC * i:BPC * (i + 1)]
        # [b, c, (p k), w] -> [b, p, c, k, w]  (h = 2p + k), fp16
        xs = np.ascontiguousarray(
            xs.reshape(BPC, C, 128, 2, W).transpose(0, 2, 1, 3, 4),
            dtype=np.float16,
        )
        m = {"x": xs}
        m.update(consts)
        in_maps.append(m)
    nc = _get_nc()
    res = run_bass_kernel_spmd(nc, in_maps, core_ids=list(range(N_CORES)), trace=trace)
    outs = []
    for i in range(N_CORES):
        o = res.results[i]["out"]  # [b, p, c, k, w] fp16
        o = o.transpose(0, 2, 1, 3, 4).reshape(BPC, C, H, W).astype(np.float32)
        outs.append(o)
    out = np.concatenate(outs, axis=0)
    return out, res


def kernel(x, w1, b1, w2, b2):
    out, _ = _execute({"x": x, "w1": w1, "b1": b1, "w2": w2, "b2": b2}, trace=False)
    return out


# revision 17
# speedup vs baseline: 1.0624x; 1.0467x over previous
"""Trainium2 Bass kernel for nn_CALayer (FFT-magnitude channel attention).

Math per (b, c) image X [256, 256] (real):
  F(p, q) = 2D DFT;  y[b,c] = mean over the centered (fftshifted) 100x100
  low-frequency crop of |F|;  s = sigmoid(w2 @ relu(w1 @ y + b1) + b2);
  out = x * s[:, :, None, None].

Implementation: DFT-as-matmul with Hermitian reduction. Since X is real,
|F(-p,-q)| = |F(p,q)|, so only p in 0..50 (51 rows) and q in -50..50
(101 cols) of the spectrum are computed, and the crop sum over
p,q in [-50, 49]^2 is recovered as two q-window sums:
  S = sum_{q in -50..49} sum_{p in 0..49} |F| + sum_{q in -49..50} sum_{p in 1..50} |F|.

Memory-regime design (v2):
  - x is host-pre-transposed to [b, p, c, k, w] (h = 2p + k) so every
    input DMA descriptor moves 16 KiB contiguous per partition.
  - x lands in SBUF as f32, is converted once to an fp16 resident copy
    (both batches fully resident: 16 group tiles x 8 KiB/partition).
    The fp16 copy feeds the FFT matmuls AND is scaled in place and
    DMA'd out as fp16 (output HBM traffic halved); the host converts
    back to f32. Total per-core HBM traffic 50.3 MB ~= 141 us roofline.
  - input DMAs ride the sync-engine HW queue, output DMAs the
    activation-engine HW queue, so neither blocks the other, and the
    input stream never waits on batch-0 writeback (full residency).

Dataflow per unit (batch b, channel-group g of 8):
  - step A (PE): U^T[w, n] = X^T @ Wu per channel (h = 2p+k indexing)
  - step B (PE): F^T[q, (ch, p)] = Wv^T @ U^T  (4 real matmuls per part)
  - mag (ACT/DVE): sqrt(Fr^2 + Fi^2) -> fp16
  - crop sum: both q-window indicator matmuls accumulate into one PSUM
    row (PE), one windowed free-dim reduce (DVE) -> y[1, 64] slice
  - after 8 units: SE block on-device; s broadcast via ones-matmul
  - in-place fp16 per-channel scale of the resident tiles, DMA out

Sharding: pure data parallel over batch: core i handles batches 2i, 2i+1.
"""

import os
import sys

for _p in (
    "/root/.axon_site",
    "/root/.axon_site/_ro/trn_rl_repo",
    "/root/.axon_site/_ro/pypackages",
    "/opt/trn_rl_repo",
):
    if os.path.isdir(_p) and _p not in sys.path:
        sys.path.append(_p)

import numpy as np

import concourse.bacc as bacc
import concourse.mybir as mybir
import concourse.tile as tile
from concourse.bass_utils import run_bass_kernel_spmd

N_CORES = 8
B, C, H, W = 16, 64, 256, 256
BPC = B // N_CORES  # batches per core
CROP = 50
NP_ = 51   # p = 0..50
NQ = 101   # q = -50..50
GS = 8     # channels per group
NG = C // GS
NU = 104   # u cols per w-half: 51 cos + pad + 51 (-sin) + pad
F32 = mybir.dt.float32
F16 = mybir.dt.float16
AF = mybir.ActivationFunctionType


def _build_consts(w1, b1, w2, b2):
    h_idx = np.arange(H)
    ang_p = 2 * np.pi * np.outer(h_idx, np.arange(NP_)) / H
    wu = np.zeros((H, NU), np.float32)
    wu[:, 0:NP_] = np.cos(ang_p)             # cols 0..50, col 51 zero pad
    wu[:, 52:52 + NP_] = -np.sin(ang_p)      # cols 52..102, col 103 zero pad
    wu2 = wu.reshape(128, 2, NU)             # [p, k, n] with h = 2p + k
    ang_q = 2 * np.pi * np.outer(h_idx, np.arange(-CROP, CROP + 1)) / W
    cq = np.cos(ang_q).astype(np.float32)
    sq = np.sin(ang_q).astype(np.float32)
    wv = np.concatenate([cq, sq, -sq], axis=1)      # [256, 303]
    wv2 = wv.reshape(2, 128, 303).transpose(1, 0, 2)  # [p, k, :] with w = 128k + p
    r1 = np.zeros((NQ, 2), np.float32)
    r1[0:100, 0] = 1.0  # q in -50..49
    r1[1:101, 1] = 1.0  # q in -49..50
    return {
        "wu": np.ascontiguousarray(wu2).astype(np.float16),
        "wv": np.ascontiguousarray(wv2).astype(np.float16),
        "r1ind": r1.astype(np.float16),
        "id1": np.ones((1, 1), np.float32),
        "ones128": np.ones((1, 128), np.float32),
        "w1t": np.ascontiguousarray(w1.T.astype(np.float32) / 1e4),  # fold /10000
        "b1c": np.ascontiguousarray(b1.astype(np.float32).reshape(-1, 1)),
        "w2t": np.ascontiguousarray(w2.T.astype(np.float32)),
        "b2r": np.ascontiguousarray(b2.astype(np.float32).reshape(1, -1)),
    }


def _build_nc():
    nc = bacc.Bacc("TRN2", target_bir_lowering=False, debug=False)
    # x pre-transposed AND pre-converted to fp16 on host:
    # [b, p, c, k, w] with h = 2p + k
    x_d = nc.dram_tensor("x", [BPC, 128, C, 2, W], F16, kind="ExternalInput").ap()
    out_d = nc.dram_tensor("out", [BPC, 128, C, 2, W], F16, kind="ExternalOutput").ap()
    wu_d = nc.dram_tensor("wu", [128, 2, NU], F16, kind="ExternalInput").ap()
    wv_d = nc.dram_tensor("wv", [128, 2, 303], F16, kind="ExternalInput").ap()
    r1_d = nc.dram_tensor("r1ind", [NQ, 2], F16, kind="ExternalInput").ap()
    id1_d = nc.dram_tensor("id1", [1, 1], F32, kind="ExternalInput").ap()
    ones128_d = nc.dram_tensor("ones128", [1, 128], F32, kind="ExternalInput").ap()
    w1t_d = nc.dram_tensor("w1t", [C, 4], F32, kind="ExternalInput").ap()
    b1c_d = nc.dram_tensor("b1c", [4, 1], F32, kind="ExternalInput").ap()
    w2t_d = nc.dram_tensor("w2t", [4, C], F32, kind="ExternalInput").ap()
    b2r_d = nc.dram_tensor("b2r", [1, C], F32, kind="ExternalInput").ap()

    with tile.TileContext(nc) as tc:
        with (
            tc.tile_pool(name="consts", bufs=1) as cpool,
            tc.tile_pool(name="xr", bufs=2 * NG) as xpool,
            tc.tile_pool(name="work", bufs=2) as wpool,
            tc.tile_pool(name="psA", bufs=2, space="PSUM") as pA,
            tc.tile_pool(name="psB", bufs=2, space="PSUM") as pB,
            tc.tile_pool(name="psS", bufs=1, space="PSUM") as pS,
        ):
            wu_sb = cpool.tile([128, 2, NU], F16, name="wu_sb")
            nc.sync.dma_start(wu_sb[:], wu_d[:])
            wv_sb = cpool.tile([128, 2, 303], F16, name="wv_sb")
            nc.sync.dma_start(wv_sb[:], wv_d[:])
            r1_sb = cpool.tile([NQ, 2], F16, name="r1_sb")
            nc.sync.dma_start(r1_sb[:], r1_d[:])
            id1_sb = cpool.tile([1, 1], F32, name="id1_sb")
            nc.sync.dma_start(id1_sb[:], id1_d[:])
            ones128_sb = cpool.tile([1, 128], F32, name="ones128_sb")
            nc.sync.dma_start(ones128_sb[:], ones128_d[:])
            w1t_sb = cpool.tile([C, 4], F32, name="w1t_sb")
            nc.sync.dma_start(w1t_sb[:], w1t_d[:])
            b1c_sb = cpool.tile([4, 1], F32, name="b1c_sb")
            nc.sync.dma_start(b1c_sb[:], b1c_d[:])
            w2t_sb = cpool.tile([4, C], F32, name="w2t_sb")
            nc.sync.dma_start(w2t_sb[:], w2t_d[:])
            b2r_sb = cpool.tile([1, C], F32, name="b2r_sb")
            nc.sync.dma_start(b2r_sb[:], b2r_d[:])

            xrs = {}
            ys = {}
            sbs = {}

            def emit_unit(b, g):
                xr = xpool.tile([128, GS, 2, W], F16, name="xr", tag="xr")
                nc.sync.dma_start(xr[:], x_d[b, :, GS * g:GS * (g + 1)])
                xrs[(b, g)] = xr

                if g == 0:
                    ys[b] = wpool.tile([1, C], F32, name="y_sb", tag="y")
                y_sb = ys[b]

                # ---- step A: U^T = X^T @ Wu per channel (both w-chunks),
                # PSUM batched over pairs of channels to halve copy count
                u_sb = wpool.tile([128, GS, 2, NU], F16, name="u_sb", tag="u")
                for jb in range(GS // 2):
                    psA = pA.tile([128, 2, 2, NU], F32, name="psA", tag="uA")
                    for jj in range(2):
                        j = 2 * jb + jj
                        for wk in range(2):
                            for kk in range(2):
                                nc.tensor.matmul(
                                    psA[:, jj, wk, :],
                                    xr[:, j, kk, 128 * wk:128 * (wk + 1)],
                                    wu_sb[:, kk, :],
                                    start=(kk == 0),
                                    stop=(kk == 1),
                                )
                    dst = u_sb[:, 2 * jb:2 * jb + 2]
                    if jb == 3:
                        nc.scalar.copy(dst, psA[:])
                    else:
                        nc.vector.tensor_copy(dst, psA[:])

                # ---- step B: F^T[q, (ch, p)] with complex arithmetic
                psB = pB.tile([NQ, 1024], F32, name="psB", tag="fB")
                fr = psB[:, 0:416]
                fi = psB[:, 512:928]
                fr_terms, fi_terms = [], []
                for k in range(2):
                    ur = u_sb[:, :, k, 0:52]
                    ui = u_sb[:, :, k, 52:104]
                    ck = wv_sb[:, k, 0:101]
                    sk = wv_sb[:, k, 101:202]
                    snk = wv_sb[:, k, 202:303]
                    fr_terms += [(ck, ur), (sk, ui)]
                    fi_terms += [(ck, ui), (snk, ur)]
                for i, (lhsT, rhs) in enumerate(fr_terms):
                    nc.tensor.matmul(fr, lhsT, rhs, start=(i == 0), stop=(i == 3))
                for i, (lhsT, rhs) in enumerate(fi_terms):
                    nc.tensor.matmul(fi, lhsT, rhs, start=(i == 0), stop=(i == 3))

                # ---- |F| = sqrt(Fr^2 + Fi^2), fp16: one ACT square over the
                # (fr, fi) pair, add on gpsimd (SBUF only), sqrt on ACT
                m2p = wpool.tile([NQ, 2, 416], F32, name="m2p", tag="m2p")
                pair = psB.rearrange("p (a x) -> p a x", a=2)[:, :, 0:416]
                nc.scalar.square(m2p[:], pair)
                nc.vector.tensor_add(m2p[:, 0], m2p[:, 0], m2p[:, 1])
                mag = wpool.tile([NQ, 416], F16, name="mag", tag="mag")
                nc.scalar.sqrt(mag[:], m2p[:, 0])

                # ---- crop sum: both q-window matmuls accumulate in PSUM,
                # then one windowed free-dim reduce -> y row slice
                mag3 = mag.rearrange("p (c x) -> p c x", c=GS)
                g2 = pS.tile([1, 400], F32, name="g2", tag="G")
                nc.tensor.matmul(
                    g2[:], r1_sb[:, 0:1], mag3[:, :, 0:50], start=True, stop=False
                )
                nc.tensor.matmul(
                    g2[:], r1_sb[:, 1:2], mag3[:, :, 1:51], start=False, stop=True
                )
                ga = g2.rearrange("p (c x) -> p c x", c=GS)
                nc.vector.reduce_sum(
                    y_sb[0:1, GS * g:GS * (g + 1)], ga, axis=mybir.AxisListType.X
                )

            def emit_se(b):
                # ---- SE block (y is pre-divided by 1e4 via w1t folding)
                y_sb = ys[b]
                yT_ps = pS.tile([C, 1], F32, name="yT_ps", tag="se")
                nc.tensor.transpose(yT_ps[:], y_sb[:], id1_sb[:])
                y_col = wpool.tile([C, 1], F32, name="y_col", tag="se2")
                nc.scalar.copy(y_col[:], yT_ps[:])
                h_ps = pS.tile([4, 1], F32, name="h_ps", tag="se")
                nc.tensor.matmul(h_ps[:], w1t_sb[:], y_col[:], start=True, stop=True)
                h_sb = wpool.tile([4, 1], F32, name="h_sb", tag="se3")
                nc.scalar.activation(h_sb[:], h_ps[:], AF.Relu, bias=b1c_sb[:])
                sarg_ps = pS.tile([1, C], F32, name="sarg_ps", tag="se")
                nc.tensor.matmul(sarg_ps[:], h_sb[:], w2t_sb[:], start=True, stop=True)
                sarg_sb = wpool.tile([1, C], F32, name="sarg_sb", tag="se4")
                nc.vector.tensor_add(sarg_sb[:], sarg_ps[:], b2r_sb[:])
                s_row = wpool.tile([1, C], F32, name="s_row", tag="se5")
                nc.scalar.activation(s_row[:], sarg_sb[:], AF.Sigmoid)
                sb_ps = pS.tile([128, C], F32, name="sb_ps", tag="se")
                nc.tensor.matmul(
                    sb_ps[:], ones128_sb[:], s_row[:], start=True, stop=True
                )
                s_b = wpool.tile([128, C], F32, name="s_b", tag="se6")
                nc.vector.tensor_copy(s_b[:], sb_ps[:])
                s_h = wpool.tile([128, C], F16, name="s_h", tag="se7")
                nc.scalar.copy(s_h[:], sb_ps[:])
                sbs[b] = (s_b, s_h)

            def emit_scale(b, g):
                # in-place fp16 scale: one 4-channel broadcast multiply on
                # DVE, a 2-channel one on gpsimd, 2 per-channel muls on ACT
                xr = xrs[(b, g)]
                s_b, s_h = sbs[b]
                sg1 = (
                    s_h[:, GS * g:GS * g + 4]
                    .unsqueeze(-1).unsqueeze(-1).broadcast_to([128, 4, 2, W])
                )
                nc.vector.tensor_mul(xr[:, 0:4], xr[:, 0:4], sg1)
                sg2 = (
                    s_h[:, GS * g + 4:GS * g + 6]
                    .unsqueeze(-1).unsqueeze(-1).broadcast_to([128, 2, 2, W])
                )
                nc.gpsimd.tensor_mul(xr[:, 4:6], xr[:, 4:6], sg2)
                for j in range(6, GS):
                    sc = s_b[:, GS * g + j:GS * g + j + 1]
                    nc.scalar.mul(xr[:, j], xr[:, j], sc)

            def emit_out(b, g, eng):
                eng.dma_start(out_d[b, :, GS * g:GS * (g + 1)], xrs[(b, g)][:])

            # Schedule: batch-0 SE right after its last unit; batch-0 scales
            # and writebacks interleaved with batch-1 units (lagged so the
            # in-order engine streams never stall on not-yet-ready sems);
            # batch-0 outs ride the ACT HW queue (overlap b1 input), batch-1
            # outs ride the sync queue (input long done by then).
            units = [(b, g) for b in range(BPC) for g in range(NG)]
            for u, (b, g) in enumerate(units):
                emit_unit(b, g)
                if u == NG - 1:
                    emit_se(0)
                if NG + 1 <= u <= NG + 4:
                    gg = 2 * (u - NG - 1)
                    emit_scale(0, gg)
                    emit_scale(0, gg + 1)
                if NG + 3 <= u <= NG + 6:
                    gg = 2 * (u - NG - 3)
                    emit_out(0, gg, nc.scalar)
                    emit_out(0, gg + 1, nc.scalar)
            emit_se(1)
            for g in range(NG):
                emit_scale(1, g)
                emit_out(1, g, nc.scalar)

    nc.compile()
    return nc


_NC = None


def _get_nc():
    global _NC
    if _NC is None:
        _NC = _build_nc()
    return _NC


def _execute(inputs, trace=False):
    x = np.asarray(inputs["x"], dtype=np.float32)
    consts = _build_consts(
        np.asarray(inputs["w1"]), np.asarray(inputs["b1"]),
        np.asarray(inputs["w2"]), np.asarray(inputs["b2"]),
    )
    in_maps = []
    for i in range(N_CORES):
        xs = x[BPC * i:BPC * (i + 1)]
        # [b, c, (p k), w] -> [b, p, c, k, w]  (h = 2p + k), fp16
        xs = np.ascontiguousarray(
            xs.reshape(BPC, C, 128, 2, W).transpose(0, 2, 1, 3, 4),
            dtype=np.float16,
        )
        m = {"x": xs}
        m.update(consts)
        in_maps.append(m)
    nc = _get_nc()
    res = run_bass_kernel_spmd(nc, in_maps, core_ids=list(range(N_CORES)), trace=trace)
    outs = []
    for i in range(N_CORES):
        o = res.results[i]["out"]  # [b, p, c, k, w] fp16
        o = o.transpose(0, 2, 1, 3, 4).reshape(BPC, C, H, W).astype(np.float32)
        outs.append(o)
    out = np.concatenate(outs, axis=0)
    return out, res


def kernel(x, w1, b1, w2, b2):
    out, _ = _execute({"x": x, "w1": w1, "b1": b1, "w2": w2, "b2": b2}, trace=False)
    return out


# revision 20
# speedup vs baseline: 1.2578x; 1.1840x over previous
"""Trainium2 Bass kernel for nn_CALayer (FFT-magnitude channel attention).

Math per (b, c) image X [256, 256] (real):
  F(p, q) = 2D DFT;  y[b,c] = mean over the centered (fftshifted) 100x100
  low-frequency crop of |F|;  s = sigmoid(w2 @ relu(w1 @ y + b1) + b2);
  out = x * s[:, :, None, None].

Implementation: DFT-as-matmul with Hermitian reduction. Since X is real,
|F(-p,-q)| = |F(p,q)|, so only p in 0..50 (51 rows) and q in -50..50
(101 cols) of the spectrum are computed, and the crop sum over
p,q in [-50, 49]^2 is recovered as two q-window sums:
  S = sum_{q in -50..49} sum_{p in 0..49} |F| + sum_{q in -49..50} sum_{p in 1..50} |F|.

Memory-regime design (v2):
  - x is host-pre-transposed to [b, p, c, k, w] (h = 2p + k) so every
    input DMA descriptor moves 16 KiB contiguous per partition.
  - x lands in SBUF as f32, is converted once to an fp16 resident copy
    (both batches fully resident: 16 group tiles x 8 KiB/partition).
    The fp16 copy feeds the FFT matmuls AND is scaled in place and
    DMA'd out as fp16 (output HBM traffic halved); the host converts
    back to f32. Total per-core HBM traffic 50.3 MB ~= 141 us roofline.
  - input DMAs ride the sync-engine HW queue, output DMAs the
    activation-engine HW queue, so neither blocks the other, and the
    input stream never waits on batch-0 writeback (full residency).

Dataflow per unit (batch b, channel-group g of 8):
  - step A (PE): U^T[w, n] = X^T @ Wu per channel (h = 2p+k indexing)
  - step B (PE): F^T[q, (ch, p)] = Wv^T @ U^T  (4 real matmuls per part)
  - mag (ACT/DVE): sqrt(Fr^2 + Fi^2) -> fp16
  - crop sum: both q-window indicator matmuls accumulate into one PSUM
    row (PE), one windowed free-dim reduce (DVE) -> y[1, 64] slice
  - after 8 units: SE block on-device; s broadcast via ones-matmul
  - in-place fp16 per-channel scale of the resident tiles, DMA out

Sharding: pure data parallel over batch: core i handles batches 2i, 2i+1.
"""

import os
import sys

for _p in (
    "/root/.axon_site",
    "/root/.axon_site/_ro/trn_rl_repo",
    "/root/.axon_site/_ro/pypackages",
    "/opt/trn_rl_repo",
):
    if os.path.isdir(_p) and _p not in sys.path:
        sys.path.append(_p)

import numpy as np

import concourse.bacc as bacc
import concourse.mybir as mybir
import concourse.tile as tile
from concourse.bass_utils import run_bass_kernel_spmd

N_CORES = 8
B, C, H, W = 16, 64, 256, 256
BPC = B // N_CORES  # batches per core
CROP = 50
NP_ = 51   # p = 0..50
NQ = 101   # q = -50..50
GS = 8     # channels per group
NG = C // GS
NU = 104   # u cols per w-half: 51 cos + pad + 51 (-sin) + pad
F32 = mybir.dt.float32
F16 = mybir.dt.float16
AF = mybir.ActivationFunctionType


def _build_consts(w1, b1, w2, b2):
    h_idx = np.arange(H)
    ang_p = 2 * np.pi * np.outer(h_idx, np.arange(NP_)) / H
    wu = np.zeros((H, NU), np.float32)
    wu[:, 0:NP_] = np.cos(ang_p)             # cols 0..50, col 51 zero pad
    wu[:, 52:52 + NP_] = -np.sin(ang_p)      # cols 52..102, col 103 zero pad
    wu2 = wu.reshape(128, 2, NU)             # [p, k, n] with h = 2p + k
    ang_q = 2 * np.pi * np.outer(h_idx, np.arange(-CROP, CROP + 1)) / W
    cq = np.cos(ang_q).astype(np.float32)
    sq = np.sin(ang_q).astype(np.float32)
    wv = np.concatenate([cq, sq, -sq], axis=1)      # [256, 303]
    wv2 = wv.reshape(2, 128, 303).transpose(1, 0, 2)  # [p, k, :] with w = 128k + p
    r1 = np.zeros((NQ, 2), np.float32)
    r1[0:100, 0] = 1.0  # q in -50..49
    r1[1:101, 1] = 1.0  # q in -49..50
    return {
        "wu": np.ascontiguousarray(wu2).astype(np.float16),
        "wv": np.ascontiguousarray(wv2).astype(np.float16),
        "r1ind": r1.astype(np.float16),
        "id1": np.ones((1, 1), np.float32),
        "ones128": np.ones((1, 128), np.float32),
        "w1t": np.ascontiguousarray(w1.T.astype(np.float32) / 1e4),  # fold /10000
        "b1c": np.ascontiguousarray(b1.astype(np.float32).reshape(-1, 1)),
        "w2t": np.ascontiguousarray(w2.T.astype(np.float32)),
        "b2r": np.ascontiguousarray(b2.astype(np.float32).reshape(1, -1)),
    }


def _build_nc():
    nc = bacc.Bacc("TRN2", target_bir_lowering=False, debug=False)
    # x pre-transposed AND pre-converted to fp16 on host:
    # [b, p, c, k, w] with h = 2p + k
    x_d = nc.dram_tensor("x", [BPC, 128, C, 2, W], F16, kind="ExternalInput").ap()
    out_d = nc.dram_tensor("out", [BPC, 128, C, 2, W], F16, kind="ExternalOutput").ap()
    wu_d = nc.dram_tensor("wu", [128, 2, NU], F16, kind="ExternalInput").ap()
    wv_d = nc.dram_tensor("wv", [128, 2, 303], F16, kind="ExternalInput").ap()
    r1_d = nc.dram_tensor("r1ind", [NQ, 2], F16, kind="ExternalInput").ap()
    id1_d = nc.dram_tensor("id1", [1, 1], F32, kind="ExternalInput").ap()
    ones128_d = nc.dram_tensor("ones128", [1, 128], F32, kind="ExternalInput").ap()
    w1t_d = nc.dram_tensor("w1t", [C, 4], F32, kind="ExternalInput").ap()
    b1c_d = nc.dram_tensor("b1c", [4, 1], F32, kind="ExternalInput").ap()
    w2t_d = nc.dram_tensor("w2t", [4, C], F32, kind="ExternalInput").ap()
    b2r_d = nc.dram_tensor("b2r", [1, C], F32, kind="ExternalInput").ap()

    with tile.TileContext(nc) as tc:
        with (
            tc.tile_pool(name="consts", bufs=1) as cpool,
            tc.tile_pool(name="xr", bufs=2 * NG) as xpool,
            tc.tile_pool(name="work", bufs=2) as wpool,
            tc.tile_pool(name="psA", bufs=2, space="PSUM") as pA,
            tc.tile_pool(name="psB", bufs=2, space="PSUM") as pB,
            tc.tile_pool(name="psS", bufs=1, space="PSUM") as pS,
        ):
            wu_sb = cpool.tile([128, 2, NU], F16, name="wu_sb")
            nc.sync.dma_start(wu_sb[:], wu_d[:])
            wv_sb = cpool.tile([128, 2, 303], F16, name="wv_sb")
            nc.sync.dma_start(wv_sb[:], wv_d[:])
            r1_sb = cpool.tile([NQ, 2], F16, name="r1_sb")
            nc.sync.dma_start(r1_sb[:], r1_d[:])
            id1_sb = cpool.tile([1, 1], F32, name="id1_sb")
            nc.sync.dma_start(id1_sb[:], id1_d[:])
            ones128_sb = cpool.tile([1, 128], F32, name="ones128_sb")
            nc.sync.dma_start(ones128_sb[:], ones128_d[:])
            w1t_sb = cpool.tile([C, 4], F32, name="w1t_sb")
            nc.sync.dma_start(w1t_sb[:], w1t_d[:])
            b1c_sb = cpool.tile([4, 1], F32, name="b1c_sb")
            nc.sync.dma_start(b1c_sb[:], b1c_d[:])
            w2t_sb = cpool.tile([4, C], F32, name="w2t_sb")
            nc.sync.dma_start(w2t_sb[:], w2t_d[:])
            b2r_sb = cpool.tile([1, C], F32, name="b2r_sb")
            nc.sync.dma_start(b2r_sb[:], b2r_d[:])

            xrs = {}
            ys = {}
            us = {}
            sbs = {}

            def emit_A(b, g):
                xr = xpool.tile([128, GS, 2, W], F16, name="xr", tag="xr")
                nc.sync.dma_start(xr[:], x_d[b, :, GS * g:GS * (g + 1)])
                xrs[(b, g)] = xr

                if g == 0:
                    ys[b] = wpool.tile([1, C], F32, name="y_sb", tag="y")

                # ---- step A: U^T = X^T @ Wu per channel (both w-chunks),
                # PSUM batched over pairs of channels to halve copy count
                u_sb = wpool.tile([128, GS, 2, NU], F16, name="u_sb", tag="u", bufs=3)
                for jb in range(GS // 2):
                    psA = pA.tile([128, 2, 2, NU], F32, name="psA", tag="uA")
                    for jj in range(2):
                        j = 2 * jb + jj
                        for wk in range(2):
                            for kk in range(2):
                                nc.tensor.matmul(
                                    psA[:, jj, wk, :],
                                    xr[:, j, kk, 128 * wk:128 * (wk + 1)],
                                    wu_sb[:, kk, :],
                                    start=(kk == 0),
                                    stop=(kk == 1),
                                )
                    dst = u_sb[:, 2 * jb:2 * jb + 2]
                    if jb == 3:
                        nc.scalar.copy(dst, psA[:])
                    else:
                        nc.vector.tensor_copy(dst, psA[:])
                us[(b, g)] = u_sb

            def emit_Bcrop(b, g):
                y_sb = ys[b]
                u_sb = us[(b, g)]
                # ---- step B: F^T[q, (ch, p)] with complex arithmetic
                psB = pB.tile([NQ, 1024], F32, name="psB", tag="fB")
                fr = psB[:, 0:416]
                fi = psB[:, 512:928]
                fr_terms, fi_terms = [], []
                for k in range(2):
                    ur = u_sb[:, :, k, 0:52]
                    ui = u_sb[:, :, k, 52:104]
                    ck = wv_sb[:, k, 0:101]
                    sk = wv_sb[:, k, 101:202]
                    snk = wv_sb[:, k, 202:303]
                    fr_terms += [(ck, ur), (sk, ui)]
                    fi_terms += [(ck, ui), (snk, ur)]
                for i, (lhsT, rhs) in enumerate(fr_terms):
                    nc.tensor.matmul(fr, lhsT, rhs, start=(i == 0), stop=(i == 3))
                for i, (lhsT, rhs) in enumerate(fi_terms):
                    nc.tensor.matmul(fi, lhsT, rhs, start=(i == 0), stop=(i == 3))

                # ---- |F| = sqrt(Fr^2 + Fi^2), fp16: one ACT square over the
                # (fr, fi) pair, add on DVE, sqrt on ACT
                m2p = wpool.tile([NQ, 2, 416], F32, name="m2p", tag="m2p")
                pair = psB.rearrange("p (a x) -> p a x", a=2)[:, :, 0:416]
                nc.scalar.square(m2p[:], pair)
                nc.vector.tensor_add(m2p[:, 0], m2p[:, 0], m2p[:, 1])
                mag = wpool.tile([NQ, 416], F16, name="mag", tag="mag")
                nc.scalar.sqrt(mag[:], m2p[:, 0])

                # ---- crop sum: both q-window matmuls accumulate in PSUM,
                # then one windowed free-dim reduce -> y row slice
                mag3 = mag.rearrange("p (c x) -> p c x", c=GS)
                g2 = pS.tile([1, 400], F32, name="g2", tag="G")
                nc.tensor.matmul(
                    g2[:], r1_sb[:, 0:1], mag3[:, :, 0:50], start=True, stop=False
                )
                nc.tensor.matmul(
                    g2[:], r1_sb[:, 1:2], mag3[:, :, 1:51], start=False, stop=True
                )
                ga = g2.rearrange("p (c x) -> p c x", c=GS)
                nc.vector.reduce_sum(
                    y_sb[0:1, GS * g:GS * (g + 1)], ga, axis=mybir.AxisListType.X
                )

            def emit_se(b):
                # ---- SE block (y is pre-divided by 1e4 via w1t folding)
                y_sb = ys[b]
                yT_ps = pS.tile([C, 1], F32, name="yT_ps", tag="se")
                nc.tensor.transpose(yT_ps[:], y_sb[:], id1_sb[:])
                y_col = wpool.tile([C, 1], F32, name="y_col", tag="se2")
                nc.scalar.copy(y_col[:], yT_ps[:])
                h_ps = pS.tile([4, 1], F32, name="h_ps", tag="se")
                nc.tensor.matmul(h_ps[:], w1t_sb[:], y_col[:], start=True, stop=True)
                h_sb = wpool.tile([4, 1], F32, name="h_sb", tag="se3")
                nc.scalar.activation(h_sb[:], h_ps[:], AF.Relu, bias=b1c_sb[:])
                sarg_ps = pS.tile([1, C], F32, name="sarg_ps", tag="se")
                nc.tensor.matmul(sarg_ps[:], h_sb[:], w2t_sb[:], start=True, stop=True)
                sarg_sb = wpool.tile([1, C], F32, name="sarg_sb", tag="se4")
                nc.vector.tensor_add(sarg_sb[:], sarg_ps[:], b2r_sb[:])
                s_row = wpool.tile([1, C], F32, name="s_row", tag="se5")
                nc.scalar.activation(s_row[:], sarg_sb[:], AF.Sigmoid)
                sb_ps = pS.tile([128, C], F32, name="sb_ps", tag="se")
                nc.tensor.matmul(
                    sb_ps[:], ones128_sb[:], s_row[:], start=True, stop=True
                )
                s_b = wpool.tile([128, C], F32, name="s_b", tag="se6")
                nc.vector.tensor_copy(s_b[:], sb_ps[:])
                s_h = wpool.tile([128, C], F16, name="s_h", tag="se7")
                nc.scalar.copy(s_h[:], sb_ps[:])
                sbs[b] = (s_b, s_h)

            def emit_scale(b, g):
                # in-place fp16 scale: one 5-channel broadcast multiply on
                # DVE, 3 per-channel muls on ACT
                xr = xrs[(b, g)]
                s_b, s_h = sbs[b]
                sg1 = (
                    s_h[:, GS * g:GS * g + 5]
                    .unsqueeze(-1).unsqueeze(-1).broadcast_to([128, 5, 2, W])
                )
                nc.vector.tensor_mul(xr[:, 0:5], xr[:, 0:5], sg1)
                for j in range(5, GS):
                    sc = s_b[:, GS * g + j:GS * g + j + 1]
                    nc.scalar.mul(xr[:, j], xr[:, j], sc)

            def emit_out(b, g, eng):
                eng.dma_start(out_d[b, :, GS * g:GS * (g + 1)], xrs[(b, g)][:])

            # Schedule: the PE stream is software-pipelined one unit deep
            # (A of unit u+1 is emitted before B/crop of unit u) so the PE
            # never idles waiting for unit u's PSUM->SBUF copies. Batch-0
            # finish (SE + scale + writeback) is emitted compactly after
            # unit 9; batch-0 outs ride the ACT HW queue (overlap batch-1
            # input), batch-1 outs ride the sync queue (input done by then).
            units = [(b, g) for b in range(BPC) for g in range(NG)]
            for u, (b, g) in enumerate(units):
                emit_A(b, g)
                if u >= 1:
                    emit_Bcrop(*units[u - 1])
                if u == NG + 1:
                    emit_se(0)
                    for gg in range(NG):
                        emit_scale(0, gg)
                        emit_out(0, gg, nc.scalar)
            emit_Bcrop(*units[-1])
            emit_se(1)
            for g in range(NG):
                emit_scale(1, g)
                emit_out(1, g, nc.sync)

    nc.compile()
    return nc


_NC = None


def _get_nc():
    global _NC
    if _NC is None:
        _NC = _build_nc()
    return _NC


def _execute(inputs, trace=False):
    x = np.asarray(inputs["x"], dtype=np.float32)
    consts = _build_consts(
        np.asarray(inputs["w1"]), np.asarray(inputs["b1"]),
        np.asarray(inputs["w2"]), np.asarray(inputs["b2"]),
    )
    in_maps = []
    for i in range(N_CORES):
        xs = x[BPC * i:BPC * (i + 1)]
        # [b, c, (p k), w] -> [b, p, c, k, w]  (h = 2p + k), fp16
        xs = np.ascontiguousarray(
            xs.reshape(BPC, C, 128, 2, W).transpose(0, 2, 1, 3, 4),
            dtype=np.float16,
        )
        m = {"x": xs}
        m.update(consts)
        in_maps.append(m)
    nc = _get_nc()
    res = run_bass_kernel_spmd(nc, in_maps, core_ids=list(range(N_CORES)), trace=trace)
    outs = []
    for i in range(N_CORES):
        o = res.results[i]["out"]  # [b, p, c, k, w] fp16
        o = o.transpose(0, 2, 1, 3, 4).reshape(BPC, C, H, W).astype(np.float32)
        outs.append(o)
    out = np.concatenate(outs, axis=0)
    return out, res


def kernel(x, w1, b1, w2, b2):
    out, _ = _execute({"x": x, "w1": w1, "b1": b1, "w2": w2, "b2": b2}, trace=False)
    return out
